# revision 2
# baseline (speedup 1.0000x reference)
"""Trainium2 Bass kernel v2 for nn_AttentionBlock — PE-offloaded reductions.

Same math/sharding as kernel.py (see its docstring), but the three big
reductions move off the DVE onto the idle TensorEngine via identity-matmul
PSUM accumulation:
  - qk d-sum: per tap-pair, 16 shifted identity-MMs accumulate
    sum_d q*k products into a [128, 2*144] PSUM tile; ACT exp reads PSUM.
  - qsum: 16 shifted identity-MMs over q2.
  - AV tap-sum (AV_PE): per tap, 6 identity-MMs accumulate the weighted
    products into 6 row-strip PSUM tiles; ACT relu reads PSUM and stores.
DVE keeps only the elementwise mults + softmax normalize.
"""

import sys
import os

sys.path.insert(0, "/opt/trn_rl_repo")

import numpy as np

B, C, H, W = 4, 64, 96, 96
KS, NH = 7, 4
HALO = (KS - 1) // 2          # 3
NCORES = 8
RPC = H // 2                  # 48 rows per core
G = 2                         # partition groups per core
RPG = RPC // G                # 24 rows per group
KR = RPG + KS - 1             # 30 k/v rows per group
XR = RPC + KS - 1             # 54 x rows per core
WE = W + KS - 1               # 102 extended cols
NB = W // 16                  # 6 w-blocks
NS = RPG * NB                 # 144 sites per partition
NT = KS * KS                  # 49 taps

# feature flags
QK_PE = False     # qk d-reduce on PE (measured: strided moving too slow)
AV_PE = True      # AV tap accumulation on PE (else DVE tree)
QSUM_PE = True    # qsum on PE
DEN_PE = False    # softmax denominator on PE (f32 moving)
TAP_D = 8         # weight expansion width (16 = full, 8/4 = split mults)

_cache = {}


def _build():
    import concourse.bacc as bacc
    import concourse.bass as bass
    import concourse.tile as tile
    from concourse import mybir

    f32 = mybir.dt.float32
    f16 = mybir.dt.float16
    i32 = mybir.dt.int32
    Act = mybir.ActivationFunctionType

    nc = bacc.Bacc(
        "TRN2",
        target_bir_lowering=False,
        debug=False,
        enable_asserts=False,
        num_devices=NCORES,
    )

    xc_d = nc.dram_tensor("xc", [C + 1, XR, WE], f16, kind="ExternalInput").ap()
    wq_d = nc.dram_tensor("wq", [C + 1, C], f16, kind="ExternalInput").ap()
    wk_d = nc.dram_tensor("wk", [C + 1, C], f16, kind="ExternalInput").ap()
    wv_d = nc.dram_tensor("wv", [C + 1, C], f16, kind="ExternalInput").ap()
    rel_d = nc.dram_tensor("relv", [NT], f32, kind="ExternalInput").ap()
    eye_d = nc.dram_tensor("eyem", [128, 128], f16, kind="ExternalInput").ap()
    out_d = nc.dram_tensor("outp", [2 * C, RPG, W], f32, kind="ExternalOutput").ap()

    from contextlib import ExitStack

    with tile.TileContext(nc) as tc:
        with ExitStack() as stk:
            wp = stk.enter_context(tc.tile_pool(name="wpool", bufs=1))
            mp = stk.enter_context(tc.tile_pool(name="main", bufs=1))
            tp = stk.enter_context(tc.tile_pool(name="tmp", bufs=2))
            pp_cm = tc.tile_pool(name="psum", bufs=2, space=bass.MemorySpace.PSUM)
            pp = pp_cm.__enter__()
            xp_cm = tc.tile_pool(name="xpool", bufs=1)
            xp = xp_cm.__enter__()

            # ---- loads ----
            wq = wp.tile([C + 1, C], f16)
            wk = wp.tile([C + 1, C], f16)
            wv = wp.tile([C + 1, C], f16)
            eye = wp.tile([128, 128], f16)
            nc.sync.dma_start(out=wk, in_=wk_d)
            nc.sync.dma_start(out=wv, in_=wv_d)
            nc.sync.dma_start(out=wq, in_=wq_d)
            nc.sync.dma_start(out=eye, in_=eye_d)
            xc = xp.tile([C + 1, XR, WE], f16)
            for r0, r1 in ((0, 15), (15, 30), (30, 42), (42, XR)):
                nc.sync.dma_start(out=xc[:, r0:r1, :], in_=xc_d[:, r0:r1, :])
            relsb = wp.tile([128, NT], f32)
            nc.gpsimd.dma_start(
                out=relsb, in_=rel_d.unsqueeze(0).broadcast_to([128, NT])
            )

            # tap permutation: even-kw first (k2o/v2o off the critical
            # path). Slot i of E/F/wn holds tap PERM[i]; relsb and k/v
            # selection use the original tap id.
            PERM = [t for t in range(NT) if (t % KS) % 2 == 0] + [
                t for t in range(NT) if (t % KS) % 2 == 1
            ]

            # ---- persistent tensors ----
            VE = WE + 10  # v tiles padded so d8 block-views stay in bounds
            k2 = mp.tile([128, KR, WE], f16)    # partition = c + 64g
            v2 = mp.tile([128, KR, VE], f16)
            k2o = mp.tile([128, KR, WE], f16)   # shifted 1 col (fp16 alignment)
            v2o = mp.tile([128, KR, VE], f16)
            q2 = mp.tile([128, RPG, W], f16)
            qs = mp.tile([128, NS], f32)
            E = mp.tile([128, NS, NT], f32, tag="bigE")   # exp(logits), tap-minor
            den = mp.tile([128, NS], f32)
            rden = mp.tile([128, NS], f32)
            wn16 = mp.tile([128, NS, NT], f16)  # normalized weights, tap-minor

            # ---- projections (as baseline) ----
            KVCH = 6
            kv_n = KR * WE // KVCH  # 510
            QCH = 6
            qrows = RPG // QCH  # 4
            qn = qrows * W  # 384

            def kv_proj(dst, wgt, evac_eng):
                for ci in range(KVCH):
                    ps = pp.tile([128, 512], f32, tag="ps_kv", name="ps")
                    for g in range(G):
                        rhs = (
                            xc[:, RPG * g : RPG * g + KR, :]
                            .rearrange("p a b -> p (a b)")[:, ci * kv_n : (ci + 1) * kv_n]
                        )
                        nc.tensor.matmul(
                            ps[64 * g : 64 * g + 64, :kv_n],
                            wgt,
                            rhs,
                            start=True,
                            stop=True,
                        )
                    rows = KR // KVCH  # 5
                    dst_sl = dst[:, ci * rows : (ci + 1) * rows, :WE]
                    ps_sl = ps[:, :kv_n].rearrange("p (a b) -> p a b", b=WE)
                    if evac_eng == "v":
                        nc.vector.tensor_copy(dst_sl, ps_sl)
                    else:
                        nc.scalar.copy(dst_sl, ps_sl)

            def shift_copy(dsto, src):
                nc.scalar.copy(dsto[:, :, : WE - 1], src[:, :, 1:WE])

            kv_proj(k2, wk, "v")
            for ci in range(QCH):
                ps = pp.tile([128, 512], f32, tag="ps_q")
                for g in range(G):
                    r0 = HALO + RPG * g + ci * qrows
                    rhs = xc[:, r0 : r0 + qrows, HALO : HALO + W]
                    nc.tensor.matmul(
                        ps[64 * g : 64 * g + 64, :qn], wq, rhs, start=True, stop=True
                    )
                q2_sl = q2[:, ci * qrows : (ci + 1) * qrows, :]
                ps_sl = ps[:, :qn].rearrange("p (a b) -> p a b", b=W)
                nc.scalar.copy(q2_sl, ps_sl)
            shift_copy(k2o, k2)
            kv_proj(v2, wv, "s")
            shift_copy(v2o, v2)

            # projections emitted; release x pool address space
            xp_cm.__exit__(None, None, None)
            abp = stk.enter_context(tc.tile_pool(name="abpool", bufs=1))

            # ---- qsum ----
            q2v = q2.rearrange("p h (a b) -> p (h a) b", b=16)  # [128, 144, 16]
            if QSUM_PE:
                ps_qs = pp.tile([128, NS], f32, tag="ps_qs", bufs=1)  # 1 bank
                for d in range(16):
                    nc.tensor.matmul(
                        ps_qs, eye, q2v[:, :, d], start=(d == 0), stop=(d == 15),
                        skip_group_check=True,
                    )
                nc.scalar.copy(qs, ps_qs)
            else:
                nc.vector.reduce_sum(out=qs, in_=q2v, axis=mybir.AxisListType.X)

            pp_cm.__exit__(None, None, None)
            app = stk.enter_context(
                tc.tile_pool(name="avpsum", bufs=1, space=bass.MemorySpace.PSUM)
            )

            # ---- F_t = exp(qs*rel_t), one op per tap (ACT) ----
            F = mp.tile([128, NS, NT], f32, tag="bigF")  # tap-minor
            for i in range(NT):
                t = PERM[i]
                nc.scalar.activation(
                    F[:, :, i], qs, Act.Exp, scale=relsb[:, t : t + 1]
                )

            # ---- qk taps ----
            def ksl(kh, kw):
                s, o = (k2, kw) if kw % 2 == 0 else (k2o, kw - 1)
                return s[:, kh : kh + RPG, o : o + W]

            if QK_PE:
                # pairs of taps; products -> 16 shifted identity-MMs -> psum
                # logits; exp(psum) -> E (fp32 in SBUF)
                for t0 in range(0, NT, 2):
                    nb = min(2, NT - t0)
                    pr = tp.tile([128, 2, RPG, W], f16, tag="pr", bufs=3)
                    for i in range(nb):
                        t = t0 + i
                        nc.vector.tensor_mul(pr[:, i], q2, ksl(t // KS, t % KS))
                    prv = pr.rearrange("p t h (a b) -> p t (h a) b", b=16)
                    psl = pp.tile([128, 2, NS], f32, tag="ps_l", bufs=4)
                    for d in range(16):
                        nc.tensor.matmul(
                            psl[:, :nb, :], eye, prv[:, :nb, :, d],
                            start=(d == 0), stop=(d == 15),
                            skip_group_check=True,
                        )
                    nc.scalar.activation(E[:, t0 : t0 + nb, :], psl[:, :nb, :], Act.Exp)
            else:
                QB = 4
                for t0 in range(0, NT, QB):
                    nb = min(QB, NT - t0)
                    pr = tp.tile([128, QB, RPG, W], f16, tag="pr4", bufs=1)
                    for i in range(nb):
                        t = PERM[t0 + i]
                        nc.vector.tensor_mul(pr[:, i], q2, ksl(t // KS, t % KS))
                    prv = pr[:, :nb].rearrange("p t h (a b) -> p t (h a) b", b=16)
                    t1 = tp.tile([128, QB, NS, 8], f16, tag="t1", bufs=1)
                    nc.vector.tensor_add(t1[:, :nb], prv[:, :, :, 0:8], prv[:, :, :, 8:16])
                    t2 = tp.tile([128, QB, NS, 4], f16, tag="t2", bufs=1)
                    nc.vector.tensor_add(t2[:, :nb], t1[:, :nb, :, 0:4], t1[:, :nb, :, 4:8])
                    t3 = tp.tile([128, QB, NS, 2], f16, tag="t3", bufs=1)
                    nc.vector.tensor_add(t3[:, :nb], t2[:, :nb, :, 0:2], t2[:, :nb, :, 2:4])
                    Aout = E[:, :, t0 : t0 + nb].rearrange("p s t -> p t s")
                    nc.vector.tensor_add(
                        Aout, t3[:, :nb, :, 0], t3[:, :nb, :, 1]
                    )
            # ---- softmax normalize (tap-minor): exp; E *= F; den; rden; wn ----
            QS4 = NS // 4
            for q0 in range(0, NS, QS4):
                Esl = E[:, q0 : q0 + QS4, :]
                nc.scalar.activation(Esl, Esl, Act.Exp)
                nc.vector.tensor_mul(Esl, Esl, F[:, q0 : q0 + QS4, :])
                nc.vector.reduce_sum(
                    out=den[:, q0 : q0 + QS4], in_=Esl, axis=mybir.AxisListType.X
                )
                nc.vector.reciprocal(rden[:, q0 : q0 + QS4], den[:, q0 : q0 + QS4])
                nc.vector.tensor_mul(
                    wn16[:, q0 : q0 + QS4, :],
                    Esl,
                    rden[:, q0 : q0 + QS4].unsqueeze(2).broadcast_to([128, QS4, NT]),
                )

            # ---- AV phase ----
            ND = 16 // TAP_D  # mults per tap
            NSTRIP = 6
            SR = RPG // NSTRIP  # rows per strip

            if AV_PE:
                avps = [
                    app.tile([128, SR, W], f32, tag=f"avps{s}", name=f"avps{s}", bufs=1)
                    for s in range(NSTRIP)
                ]

            carry = {}
            state = {"acc": None}

            def tree_push(p, level=0):
                while level in carry and level < 3:
                    prev = carry.pop(level)
                    s = abp.tile(
                        [128, RPG, W], f16,
                        tag=f"ts{level}", name=f"ts{level}",
                        bufs=3 if level == 2 else 2,
                    )
                    nc.vector.tensor_add(s, prev, p)
                    p = s
                    level += 1
                if level == 3:
                    if state["acc"] is None:
                        state["acc"] = p
                    else:
                        nc.vector.tensor_add(state["acc"], state["acc"], p)
                else:
                    carry[level] = p

            for t0 in range(0, NT, 2):
                nb = min(2, NT - t0)
                # weight expansion to TAP_D (ACT; pair 0 on DVE, its idle slot)
                wexp = abp.tile(
                    [128, 2, RPG, NB, TAP_D], f16, tag="wexp", name="wexp", bufs=3
                )
                wsl = (
                    wn16[:, :, t0 : t0 + nb]
                    .rearrange("p (h a) t -> p t h a", a=NB)
                    .unsqueeze(4)
                    .broadcast_to([128, nb, RPG, NB, TAP_D])
                )
                if t0 == 0:
                    nc.vector.tensor_copy(wexp[:, :nb], wsl)
                else:
                    nc.scalar.copy(wexp[:, :nb], wsl)
                for i in range(nb):
                    slot = t0 + i
                    t = PERM[slot]
                    kh, kw = t // KS, t % KS
                    vsrc, kwoff = (v2, kw) if kw % 2 == 0 else (v2o, kw - 1)
                    p = abp.tile([128, RPG, NB, 16], f16, tag="avp", name="avp", bufs=3)
                    for j in range(ND):
                        c0 = kwoff + j * TAP_D
                        vview = (
                            vsrc[:, kh : kh + RPG, c0 : c0 + 16 * NB]
                            .rearrange("p h (a b) -> p h a b", b=16)[:, :, :, 0:TAP_D]
                        )
                        nc.vector.tensor_mul(
                            p[:, :, :, j * TAP_D : (j + 1) * TAP_D],
                            wexp[:, i],
                            vview,
                        )
                    if AV_PE:
                        pv = p.rearrange("p h a b -> p h (a b)")
                        for s in range(NSTRIP):
                            nc.tensor.matmul(
                                avps[s], eye,
                                pv[:, s * SR : (s + 1) * SR, :],
                                start=(slot == 0), stop=(slot == NT - 1),
                                skip_group_check=True,
                            )
                    else:
                        tree_push(p.rearrange("p h a b -> p h (a b)"))

            if AV_PE:
                for s in range(NSTRIP):
                    oute = abp.tile([128, SR, W], f32, tag="oute", name="oute", bufs=3)
                    if s % 2 == 1:
                        nc.vector.tensor_scalar_max(oute, avps[s], 0.0)
                    else:
                        nc.scalar.activation(oute, avps[s], Act.Relu)
                    nc.sync.dma_start(
                        out=out_d[:, s * SR : (s + 1) * SR, :], in_=oute
                    )
            else:
                acc = state["acc"]
                for lv in sorted(carry):
                    nc.vector.tensor_add(acc, acc, carry.pop(lv))
                oute = mp.tile([128, RPG, W], f32, tag="bigF")
                qt = RPG // 4
                for r0 in range(0, RPG, qt):
                    nc.scalar.activation(
                        oute[:, r0 : r0 + qt, :], acc[:, r0 : r0 + qt, :], Act.Relu
                    )
                    nc.sync.dma_start(
                        out=out_d[:, r0 : r0 + qt, :], in_=oute[:, r0 : r0 + qt, :]
                    )

    nc.compile()
    return nc


def _get_nc():
    if "nc" not in _cache:
        _cache["nc"] = _build()
    return _cache["nc"]


def _prep_inputs(inputs):
    """Host-side shard prep. Returns list of 8 in_maps."""
    x = np.ascontiguousarray(np.asarray(inputs["input_x"], dtype=np.float32))
    qw = np.asarray(inputs["q_w"], np.float32)
    qb = np.asarray(inputs["q_b"], np.float32)
    kw_ = np.asarray(inputs["k_w"], np.float32)
    kb = np.asarray(inputs["k_b"], np.float32)
    vw = np.asarray(inputs["v_w"], np.float32)
    vb = np.asarray(inputs["v_b"], np.float32)
    rh = np.asarray(inputs["rel_h"], np.float32).sum(0)[:, 0]  # (7,)
    rw = np.asarray(inputs["rel_w"], np.float32).sum(0)[0, :]  # (7,)

    wq = np.concatenate([qw.T, qb[None, :]], axis=0).astype(np.float16)  # (65, 64)
    wk = np.concatenate([kw_.T, kb[None, :]], axis=0).astype(np.float16)
    wv = np.concatenate([vw.T, vb[None, :]], axis=0).astype(np.float16)
    relv = (rh[:, None] + rw[None, :]).reshape(-1).astype(np.float32)  # (49,)
    eyem = np.eye(128, dtype=np.float16)

    xpad = np.zeros((B, C + 1, H + 2 * HALO, W + 2 * HALO), np.float16)
    xpad[:, :C, HALO : HALO + H, HALO : HALO + W] = x
    xpad[:, C, :, :] = 1.0

    in_maps = []
    for j in range(NCORES):
        b = j // 2
        r0 = RPC * (j % 2)
        xc = np.ascontiguousarray(xpad[b, :, r0 : r0 + XR, :])  # (65, 54, 102)
        in_maps.append(
            {"xc": xc, "wq": wq, "wk": wk, "wv": wv, "relv": relv, "eyem": eyem}
        )
    return in_maps


def _assemble(results):
    y = np.empty((B, C, H, W), np.float32)
    for j in range(NCORES):
        o = results[j]["outp"]
        b = j // 2
        r0 = RPC * (j % 2)
        for g in range(G):
            y[b, :, r0 + RPG * g : r0 + RPG * (g + 1), :] = o[64 * g : 64 * g + 64]
    return y


def _install_ntff_hook():
    import types
    import antenv

    if "antenv.axon_hooks" in sys.modules:
        return
    mod = types.ModuleType("antenv.axon_hooks")
    _state = {"hook": None}
    mod.set_axon_ntff_profile_hook = lambda h: _state.__setitem__("hook", h)
    mod.get_axon_ntff_profile_hook = lambda: _state["hook"]
    sys.modules["antenv.axon_hooks"] = mod
    antenv.axon_hooks = mod
    from trn_agent_boot.trn_boot import _ntff_profile_via_ctypes

    mod.set_axon_ntff_profile_hook(_ntff_profile_via_ctypes("/opt/axon/libaxon_pjrt.so"))
    from concourse import bass_utils

    bass_utils.upload_artifacts = lambda tmpdir: tmpdir


def kernel(**inputs) -> np.ndarray:
    from concourse import bass_utils

    nc = _get_nc()
    in_maps = _prep_inputs(inputs)
    trace = bool(int(os.environ.get("KERNEL_TRACE", "0")))
    kw = {}
    if trace:
        _install_ntff_hook()
        kw["tmpdir"] = os.environ.get("KERNEL_TRACE_DIR") or None
    res = bass_utils.run_bass_kernel_spmd(
        nc, in_maps, core_ids=list(range(NCORES)), trace=trace, **kw
    )
    _cache["last_result"] = res
    return _assemble(res.results)


def kernel_sim(inputs, cores=(0,)):
    from concourse.bass_interp import CoreSim

    nc = _get_nc()
    in_maps = _prep_inputs(inputs)
    outs = {}
    for j in cores:
        sim = CoreSim(nc, trace=False, require_finite=True, require_nnan=True)
        for name, arr in in_maps[j].items():
            sim.tensor(name)[:] = arr
        sim.simulate(check_with_hw=False)
        outs[j] = np.array(sim.tensor("outp"))
    return outs


# revision 3
# speedup vs baseline: 1.1942x; 1.1942x over previous
"""Trainium2 Bass kernel for nn_AttentionBlock (sparse 7x7 windowed per-channel
attention), v2: PE-offloaded reductions.

Semantics (validated vs reference): the torch-faithful scrambled reshape makes
this, in original coordinates, a per-(b, c, h, w-block-of-16) attention:
  logits[kh,kw] = sum_{d<16} q[c,h,16w0+d] * kpad[c,h+kh,16w0+d+kw]
                  + qsum*(rh[kh]+rw[kw])
  out[c,h,16w0+d] = relu( sum_{kh,kw} softmax(logits)[kh,kw]
                          * vpad[c,h+kh,16w0+d+kw] )
where kpad/vpad = conv1x1(x)+bias inside the image and exactly bias in the pad
border (conv of zero-padded x reproduces this).

Sharding: pure data parallel over 8 cores: core j -> batch j//2, image rows
[48*(j%2), 48*(j%2)+48). Each core packs its 48 rows as 2 partition-groups of
24 rows (partition = c + 64*g) so elementwise work uses all 128 partitions.

Engine split (the kernel is elementwise-bound; binary tensor ops only run on
the DVE, at 2 elem/cycle fp16 max, so the wins come from moving every
reduction it can spare onto other engines):
  DVE:  49 qk tap mults (fp16 2x) + 4-level pairwise tree -> fp32 logits
        (tap-minor layout so the den reduce is contiguous); softmax muls
        E*=F, wn=E*rden; AV mults in d=8 halves (fp16 2x).
  PE:   q/k/v 1x1-conv matmuls; qsum via 16 shifted identity-matmuls with
        PSUM accumulation; AV tap-sum via 6 row-strip identity-matmuls per
        tap accumulating all 49 weighted products in PSUM (measured
        ~0.5ns/col for contiguous moving; LDWEIGHTS pipelines away).
        Strided-moving matmuls measured 4.3x slower, so the qk d-reduce
        stays on the DVE tree; fp16 tensor_reduce measured 1x - no good.
  ACT:  weight-broadcast expansion to d=8 (stride-0 input forces 1x, so
        half-width halves the cost; the paired DVE mults stay 2x via
        4B-aligned block views into 112-col-padded v tiles); exp; F_t =
        exp(qs*rel_t) (per-partition scale carries rel_t); psum evacs;
        relu on even strips (odd strips relu on DVE to parallelize the
        drain).
Taps are processed even-kw-first (PERM) so the odd-shifted k2o/v2o copies
(fp16 2x alignment for odd kw) are off the critical path; E/F/wn slots are
permutation-indexed. Input DMA is split across the Sync and GpSimd (SWDGE)
queues, x ahead of the later-needed weights. fp16 quantization validated
end-to-end: rel err ~2.6e-3 vs 2e-2 tolerance (HW); ~322-385us baseline ->
~262us measured (run-to-run device clock varies ~20%).
"""

import sys
import os

sys.path.insert(0, "/opt/trn_rl_repo")

import numpy as np

B, C, H, W = 4, 64, 96, 96
KS, NH = 7, 4
HALO = (KS - 1) // 2          # 3
NCORES = 8
RPC = H // 2                  # 48 rows per core
G = 2                         # partition groups per core
RPG = RPC // G                # 24 rows per group
KR = RPG + KS - 1             # 30 k/v rows per group
XR = RPC + KS - 1             # 54 x rows per core
WE = W + KS - 1               # 102 extended cols
NB = W // 16                  # 6 w-blocks
NS = RPG * NB                 # 144 sites per partition
NT = KS * KS                  # 49 taps

# feature flags
QK_PE = False     # qk d-reduce on PE (measured: strided moving too slow)
AV_PE = True      # AV tap accumulation on PE (else DVE tree)
QSUM_PE = True    # qsum on PE
DEN_PE = False    # softmax denominator on PE (f32 moving)
TAP_D = 8         # weight expansion width (16 = full, 8/4 = split mults)

_cache = {}


def _build():
    import concourse.bacc as bacc
    import concourse.bass as bass
    import concourse.tile as tile
    from concourse import mybir

    f32 = mybir.dt.float32
    f16 = mybir.dt.float16
    i32 = mybir.dt.int32
    Act = mybir.ActivationFunctionType

    nc = bacc.Bacc(
        "TRN2",
        target_bir_lowering=False,
        debug=False,
        enable_asserts=False,
        num_devices=NCORES,
    )

    xc_d = nc.dram_tensor("xc", [C + 1, XR, WE], f16, kind="ExternalInput").ap()
    wq_d = nc.dram_tensor("wq", [C + 1, C], f16, kind="ExternalInput").ap()
    wk_d = nc.dram_tensor("wk", [C + 1, C], f16, kind="ExternalInput").ap()
    wv_d = nc.dram_tensor("wv", [C + 1, C], f16, kind="ExternalInput").ap()
    rel_d = nc.dram_tensor("relv", [NT], f32, kind="ExternalInput").ap()
    eye_d = nc.dram_tensor("eyem", [128, 128], f16, kind="ExternalInput").ap()
    out_d = nc.dram_tensor("outp", [2 * C, RPG, W], f32, kind="ExternalOutput").ap()

    from contextlib import ExitStack

    with tile.TileContext(nc) as tc:
        with ExitStack() as stk:
            wp = stk.enter_context(tc.tile_pool(name="wpool", bufs=1))
            mp = stk.enter_context(tc.tile_pool(name="main", bufs=1))
            tp = stk.enter_context(tc.tile_pool(name="tmp", bufs=2))
            pp_cm = tc.tile_pool(name="psum", bufs=2, space=bass.MemorySpace.PSUM)
            pp = pp_cm.__enter__()
            xp_cm = tc.tile_pool(name="xpool", bufs=1)
            xp = xp_cm.__enter__()

            # ---- loads ----
            wq = wp.tile([C + 1, C], f16)
            wk = wp.tile([C + 1, C], f16)
            wv = wp.tile([C + 1, C], f16)
            eye = wp.tile([128, 128], f16)
            nc.sync.dma_start(out=wk, in_=wk_d)
            nc.gpsimd.dma_start(out=wq, in_=wq_d)
            xc = xp.tile([C + 1, XR, WE], f16)
            for r0, r1 in ((0, 15), (15, 30), (30, 42), (42, XR)):
                nc.sync.dma_start(out=xc[:, r0:r1, :], in_=xc_d[:, r0:r1, :])
            relsb = wp.tile([128, NT], f32)
            nc.gpsimd.dma_start(
                out=relsb, in_=rel_d.unsqueeze(0).broadcast_to([128, NT])
            )
            nc.gpsimd.dma_start(out=wv, in_=wv_d)
            nc.gpsimd.dma_start(out=eye, in_=eye_d)

            # tap permutation: even-kw first (k2o/v2o off the critical
            # path). Slot i of E/F/wn holds tap PERM[i]; relsb and k/v
            # selection use the original tap id.
            PERM = [t for t in range(NT) if (t % KS) % 2 == 0] + [
                t for t in range(NT) if (t % KS) % 2 == 1
            ]

            # ---- persistent tensors ----
            VE = WE + 10  # v tiles padded so d8 block-views stay in bounds
            k2 = mp.tile([128, KR, WE], f16)    # partition = c + 64g
            v2 = mp.tile([128, KR, VE], f16)
            k2o = mp.tile([128, KR, WE], f16)   # shifted 1 col (fp16 alignment)
            v2o = mp.tile([128, KR, VE], f16)
            q2 = mp.tile([128, RPG, W], f16)
            qs = mp.tile([128, NS], f32)
            E = mp.tile([128, NS, NT], f32, tag="bigE")   # exp(logits), tap-minor
            den = mp.tile([128, NS], f32)
            rden = mp.tile([128, NS], f32)
            wn16 = mp.tile([128, NS, NT], f16)  # normalized weights, tap-minor

            # ---- projections (as baseline) ----
            KVCH = 6
            kv_n = KR * WE // KVCH  # 510
            QCH = 6
            qrows = RPG // QCH  # 4
            qn = qrows * W  # 384

            def kv_proj(dst, wgt, evac_eng):
                for ci in range(KVCH):
                    ps = pp.tile([128, 512], f32, tag="ps_kv", name="ps")
                    for g in range(G):
                        rhs = (
                            xc[:, RPG * g : RPG * g + KR, :]
                            .rearrange("p a b -> p (a b)")[:, ci * kv_n : (ci + 1) * kv_n]
                        )
                        nc.tensor.matmul(
                            ps[64 * g : 64 * g + 64, :kv_n],
                            wgt,
                            rhs,
                            start=True,
                            stop=True,
                        )
                    rows = KR // KVCH  # 5
                    dst_sl = dst[:, ci * rows : (ci + 1) * rows, :WE]
                    ps_sl = ps[:, :kv_n].rearrange("p (a b) -> p a b", b=WE)
                    if evac_eng == "v":
                        nc.vector.tensor_copy(dst_sl, ps_sl)
                    else:
                        nc.scalar.copy(dst_sl, ps_sl)

            def shift_copy(dsto, src):
                nc.scalar.copy(dsto[:, :, : WE - 1], src[:, :, 1:WE])

            for ci in range(QCH):
                ps = pp.tile([128, 512], f32, tag="ps_q")
                for g in range(G):
                    r0 = HALO + RPG * g + ci * qrows
                    rhs = xc[:, r0 : r0 + qrows, HALO : HALO + W]
                    nc.tensor.matmul(
                        ps[64 * g : 64 * g + 64, :qn], wq, rhs, start=True, stop=True
                    )
                q2_sl = q2[:, ci * qrows : (ci + 1) * qrows, :]
                ps_sl = ps[:, :qn].rearrange("p (a b) -> p a b", b=W)
                nc.scalar.copy(q2_sl, ps_sl)
            kv_proj(k2, wk, "v")
            shift_copy(k2o, k2)
            kv_proj(v2, wv, "s")
            shift_copy(v2o, v2)

            # projections emitted; release x pool address space
            xp_cm.__exit__(None, None, None)
            abp = stk.enter_context(tc.tile_pool(name="abpool", bufs=1))

            # ---- qsum ----
            q2v = q2.rearrange("p h (a b) -> p (h a) b", b=16)  # [128, 144, 16]
            if QSUM_PE:
                ps_qs = pp.tile([128, NS], f32, tag="ps_qs", bufs=1)  # 1 bank
                for d in range(16):
                    nc.tensor.matmul(
                        ps_qs, eye, q2v[:, :, d], start=(d == 0), stop=(d == 15),
                        skip_group_check=True,
                    )
                nc.scalar.copy(qs, ps_qs)
            else:
                nc.vector.reduce_sum(out=qs, in_=q2v, axis=mybir.AxisListType.X)

            pp_cm.__exit__(None, None, None)
            app = stk.enter_context(
                tc.tile_pool(name="avpsum", bufs=1, space=bass.MemorySpace.PSUM)
            )

            # ---- F_t = exp(qs*rel_t), one op per tap (ACT) ----
            F = mp.tile([128, NS, NT], f32, tag="bigF")  # tap-minor
            for i in range(NT):
                t = PERM[i]
                nc.scalar.activation(
                    F[:, :, i], qs, Act.Exp, scale=relsb[:, t : t + 1]
                )

            # ---- qk taps ----
            def ksl(kh, kw):
                s, o = (k2, kw) if kw % 2 == 0 else (k2o, kw - 1)
                return s[:, kh : kh + RPG, o : o + W]

            if QK_PE:
                # pairs of taps; products -> 16 shifted identity-MMs -> psum
                # logits; exp(psum) -> E (fp32 in SBUF)
                for t0 in range(0, NT, 2):
                    nb = min(2, NT - t0)
                    pr = tp.tile([128, 2, RPG, W], f16, tag="pr", bufs=3)
                    for i in range(nb):
                        t = t0 + i
                        nc.vector.tensor_mul(pr[:, i], q2, ksl(t // KS, t % KS))
                    prv = pr.rearrange("p t h (a b) -> p t (h a) b", b=16)
                    psl = pp.tile([128, 2, NS], f32, tag="ps_l", bufs=4)
                    for d in range(16):
                        nc.tensor.matmul(
                            psl[:, :nb, :], eye, prv[:, :nb, :, d],
                            start=(d == 0), stop=(d == 15),
                            skip_group_check=True,
                        )
                    nc.scalar.activation(E[:, t0 : t0 + nb, :], psl[:, :nb, :], Act.Exp)
            else:
                QB = 4
                for t0 in range(0, NT, QB):
                    nb = min(QB, NT - t0)
                    pr = tp.tile([128, QB, RPG, W], f16, tag="pr4", bufs=1)
                    for i in range(nb):
                        t = PERM[t0 + i]
                        nc.vector.tensor_mul(pr[:, i], q2, ksl(t // KS, t % KS))
                    prv = pr[:, :nb].rearrange("p t h (a b) -> p t (h a) b", b=16)
                    t1 = tp.tile([128, QB, NS, 8], f16, tag="t1", bufs=1)
                    nc.vector.tensor_add(t1[:, :nb], prv[:, :, :, 0:8], prv[:, :, :, 8:16])
                    t2 = tp.tile([128, QB, NS, 4], f16, tag="t2", bufs=1)
                    nc.vector.tensor_add(t2[:, :nb], t1[:, :nb, :, 0:4], t1[:, :nb, :, 4:8])
                    t3 = tp.tile([128, QB, NS, 2], f16, tag="t3", bufs=1)
                    nc.vector.tensor_add(t3[:, :nb], t2[:, :nb, :, 0:2], t2[:, :nb, :, 2:4])
                    Aout = E[:, :, t0 : t0 + nb].rearrange("p s t -> p t s")
                    nc.vector.tensor_add(
                        Aout, t3[:, :nb, :, 0], t3[:, :nb, :, 1]
                    )
            # ---- softmax normalize (tap-minor): exp; E *= F; den; rden; wn ----
            QS4 = NS // 4
            for q0 in range(0, NS, QS4):
                Esl = E[:, q0 : q0 + QS4, :]
                nc.scalar.activation(Esl, Esl, Act.Exp)
                nc.vector.tensor_mul(Esl, Esl, F[:, q0 : q0 + QS4, :])
                nc.vector.reduce_sum(
                    out=den[:, q0 : q0 + QS4], in_=Esl, axis=mybir.AxisListType.X
                )
                nc.vector.reciprocal(rden[:, q0 : q0 + QS4], den[:, q0 : q0 + QS4])
                nc.vector.tensor_mul(
                    wn16[:, q0 : q0 + QS4, :],
                    Esl,
                    rden[:, q0 : q0 + QS4].unsqueeze(2).broadcast_to([128, QS4, NT]),
                )

            # ---- AV phase ----
            ND = 16 // TAP_D  # mults per tap
            NSTRIP = 6
            SR = RPG // NSTRIP  # rows per strip

            if AV_PE:
                avps = [
                    app.tile([128, SR, W], f32, tag=f"avps{s}", name=f"avps{s}", bufs=1)
                    for s in range(NSTRIP)
                ]

            carry = {}
            state = {"acc": None}

            def tree_push(p, level=0):
                while level in carry and level < 3:
                    prev = carry.pop(level)
                    s = abp.tile(
                        [128, RPG, W], f16,
                        tag=f"ts{level}", name=f"ts{level}",
                        bufs=3 if level == 2 else 2,
                    )
                    nc.vector.tensor_add(s, prev, p)
                    p = s
                    level += 1
                if level == 3:
                    if state["acc"] is None:
                        state["acc"] = p
                    else:
                        nc.vector.tensor_add(state["acc"], state["acc"], p)
                else:
                    carry[level] = p

            for t0 in range(0, NT, 2):
                nb = min(2, NT - t0)
                # weight expansion to TAP_D (ACT; pair 0 on DVE, its idle slot)
                wexp = abp.tile(
                    [128, 2, RPG, NB, TAP_D], f16, tag="wexp", name="wexp", bufs=3
                )
                wsl = (
                    wn16[:, :, t0 : t0 + nb]
                    .rearrange("p (h a) t -> p t h a", a=NB)
                    .unsqueeze(4)
                    .broadcast_to([128, nb, RPG, NB, TAP_D])
                )
                if t0 == 0:
                    nc.vector.tensor_copy(wexp[:, :nb], wsl)
                else:
                    nc.scalar.copy(wexp[:, :nb], wsl)
                for i in range(nb):
                    slot = t0 + i
                    t = PERM[slot]
                    kh, kw = t // KS, t % KS
                    vsrc, kwoff = (v2, kw) if kw % 2 == 0 else (v2o, kw - 1)
                    p = abp.tile([128, RPG, NB, 16], f16, tag="avp", name="avp", bufs=3)
                    for j in range(ND):
                        c0 = kwoff + j * TAP_D
                        vview = (
                            vsrc[:, kh : kh + RPG, c0 : c0 + 16 * NB]
                            .rearrange("p h (a b) -> p h a b", b=16)[:, :, :, 0:TAP_D]
                        )
                        nc.vector.tensor_mul(
                            p[:, :, :, j * TAP_D : (j + 1) * TAP_D],
                            wexp[:, i],
                            vview,
                        )
                    if AV_PE:
                        pv = p.rearrange("p h a b -> p h (a b)")
                        for s in range(NSTRIP):
                            nc.tensor.matmul(
                                avps[s], eye,
                                pv[:, s * SR : (s + 1) * SR, :],
                                start=(slot == 0), stop=(slot == NT - 1),
                                skip_group_check=True,
                            )
                    else:
                        tree_push(p.rearrange("p h a b -> p h (a b)"))

            if AV_PE:
                for s in range(NSTRIP):
                    oute = abp.tile([128, SR, W], f32, tag="oute", name="oute", bufs=3)
                    if s % 2 == 1:
                        nc.vector.tensor_scalar_max(oute, avps[s], 0.0)
                    else:
                        nc.scalar.activation(oute, avps[s], Act.Relu)
                    nc.sync.dma_start(
                        out=out_d[:, s * SR : (s + 1) * SR, :], in_=oute
                    )
            else:
                acc = state["acc"]
                for lv in sorted(carry):
                    nc.vector.tensor_add(acc, acc, carry.pop(lv))
                oute = mp.tile([128, RPG, W], f32, tag="bigF")
                qt = RPG // 4
                for r0 in range(0, RPG, qt):
                    nc.scalar.activation(
                        oute[:, r0 : r0 + qt, :], acc[:, r0 : r0 + qt, :], Act.Relu
                    )
                    nc.sync.dma_start(
                        out=out_d[:, r0 : r0 + qt, :], in_=oute[:, r0 : r0 + qt, :]
                    )

    nc.compile()
    return nc


def _get_nc():
    if "nc" not in _cache:
        _cache["nc"] = _build()
    return _cache["nc"]


def _prep_inputs(inputs):
    """Host-side shard prep. Returns list of 8 in_maps."""
    x = np.ascontiguousarray(np.asarray(inputs["input_x"], dtype=np.float32))
    qw = np.asarray(inputs["q_w"], np.float32)
    qb = np.asarray(inputs["q_b"], np.float32)
    kw_ = np.asarray(inputs["k_w"], np.float32)
    kb = np.asarray(inputs["k_b"], np.float32)
    vw = np.asarray(inputs["v_w"], np.float32)
    vb = np.asarray(inputs["v_b"], np.float32)
    rh = np.asarray(inputs["rel_h"], np.float32).sum(0)[:, 0]  # (7,)
    rw = np.asarray(inputs["rel_w"], np.float32).sum(0)[0, :]  # (7,)

    wq = np.concatenate([qw.T, qb[None, :]], axis=0).astype(np.float16)  # (65, 64)
    wk = np.concatenate([kw_.T, kb[None, :]], axis=0).astype(np.float16)
    wv = np.concatenate([vw.T, vb[None, :]], axis=0).astype(np.float16)
    relv = (rh[:, None] + rw[None, :]).reshape(-1).astype(np.float32)  # (49,)
    eyem = np.eye(128, dtype=np.float16)

    xpad = np.zeros((B, C + 1, H + 2 * HALO, W + 2 * HALO), np.float16)
    xpad[:, :C, HALO : HALO + H, HALO : HALO + W] = x
    xpad[:, C, :, :] = 1.0

    in_maps = []
    for j in range(NCORES):
        b = j // 2
        r0 = RPC * (j % 2)
        xc = np.ascontiguousarray(xpad[b, :, r0 : r0 + XR, :])  # (65, 54, 102)
        in_maps.append(
            {"xc": xc, "wq": wq, "wk": wk, "wv": wv, "relv": relv, "eyem": eyem}
        )
    return in_maps


def _assemble(results):
    y = np.empty((B, C, H, W), np.float32)
    for j in range(NCORES):
        o = results[j]["outp"]
        b = j // 2
        r0 = RPC * (j % 2)
        for g in range(G):
            y[b, :, r0 + RPG * g : r0 + RPG * (g + 1), :] = o[64 * g : 64 * g + 64]
    return y


def _install_ntff_hook():
    import types
    import antenv

    if "antenv.axon_hooks" in sys.modules:
        return
    mod = types.ModuleType("antenv.axon_hooks")
    _state = {"hook": None}
    mod.set_axon_ntff_profile_hook = lambda h: _state.__setitem__("hook", h)
    mod.get_axon_ntff_profile_hook = lambda: _state["hook"]
    sys.modules["antenv.axon_hooks"] = mod
    antenv.axon_hooks = mod
    from trn_agent_boot.trn_boot import _ntff_profile_via_ctypes

    mod.set_axon_ntff_profile_hook(_ntff_profile_via_ctypes("/opt/axon/libaxon_pjrt.so"))
    from concourse import bass_utils

    bass_utils.upload_artifacts = lambda tmpdir: tmpdir


def kernel(**inputs) -> np.ndarray:
    from concourse import bass_utils

    nc = _get_nc()
    in_maps = _prep_inputs(inputs)
    trace = bool(int(os.environ.get("KERNEL_TRACE", "0")))
    kw = {}
    if trace:
        _install_ntff_hook()
        kw["tmpdir"] = os.environ.get("KERNEL_TRACE_DIR") or None
    res = bass_utils.run_bass_kernel_spmd(
        nc, in_maps, core_ids=list(range(NCORES)), trace=trace, **kw
    )
    _cache["last_result"] = res
    return _assemble(res.results)


def kernel_sim(inputs, cores=(0,)):
    from concourse.bass_interp import CoreSim

    nc = _get_nc()
    in_maps = _prep_inputs(inputs)
    outs = {}
    for j in cores:
        sim = CoreSim(nc, trace=False, require_finite=True, require_nnan=True)
        for name, arr in in_maps[j].items():
            sim.tensor(name)[:] = arr
        sim.simulate(check_with_hw=False)
        outs[j] = np.array(sim.tensor("outp"))
    return outs


# revision 4
# speedup vs baseline: 1.1952x; 1.0008x over previous
"""Trainium2 Bass kernel for nn_AttentionBlock (sparse 7x7 windowed per-channel
attention), v2: PE-offloaded reductions.

Semantics (validated vs reference): the torch-faithful scrambled reshape makes
this, in original coordinates, a per-(b, c, h, w-block-of-16) attention:
  logits[kh,kw] = sum_{d<16} q[c,h,16w0+d] * kpad[c,h+kh,16w0+d+kw]
                  + qsum*(rh[kh]+rw[kw])
  out[c,h,16w0+d] = relu( sum_{kh,kw} softmax(logits)[kh,kw]
                          * vpad[c,h+kh,16w0+d+kw] )
where kpad/vpad = conv1x1(x)+bias inside the image and exactly bias in the pad
border (conv of zero-padded x reproduces this).

Sharding: pure data parallel over 8 cores: core j -> batch j//2, image rows
[48*(j%2), 48*(j%2)+48). Each core packs its 48 rows as 2 partition-groups of
24 rows (partition = c + 64*g) so elementwise work uses all 128 partitions.

Engine split (the kernel is elementwise-bound; binary tensor ops only run on
the DVE, at 2 elem/cycle fp16 max, so the wins come from moving every
reduction it can spare onto other engines):
  DVE:  49 qk tap mults (fp16 2x) + 4-level pairwise tree -> fp32 logits
        (tap-minor layout so the den reduce is contiguous); softmax muls
        E*=F, wn=E*rden; AV mults in d=8 halves (fp16 2x).
  PE:   q/k/v 1x1-conv matmuls; qsum via 16 shifted identity-matmuls with
        PSUM accumulation; AV tap-sum via 6 row-strip identity-matmuls per
        tap accumulating all 49 weighted products in PSUM (measured
        ~0.5ns/col for contiguous moving; LDWEIGHTS pipelines away).
        Strided-moving matmuls measured 4.3x slower, so the qk d-reduce
        stays on the DVE tree; fp16 tensor_reduce measured 1x - no good.
  ACT:  weight-broadcast expansion to d=8 (stride-0 input forces 1x, so
        half-width halves the cost; the paired DVE mults stay 2x via
        4B-aligned block views into 112-col-padded v tiles); exp; F_t =
        exp(qs*rel_t) (per-partition scale carries rel_t); psum evacs;
        relu on even strips (odd strips relu on DVE to parallelize the
        drain).
Taps are processed even-kw-first (PERM) so the odd-shifted k2o/v2o copies
(fp16 2x alignment for odd kw) are off the critical path; E/F/wn slots are
permutation-indexed. Input DMA is split across the Sync and GpSimd (SWDGE)
queues, x ahead of the later-needed weights. fp16 quantization validated
end-to-end: rel err ~2.6e-3 vs 2e-2 tolerance (HW); ~322-385us baseline ->
~262us measured (run-to-run device clock varies ~20%).
"""

import sys
import os

sys.path.insert(0, "/opt/trn_rl_repo")

import numpy as np

B, C, H, W = 4, 64, 96, 96
KS, NH = 7, 4
HALO = (KS - 1) // 2          # 3
NCORES = 8
RPC = H // 2                  # 48 rows per core
G = 2                         # partition groups per core
RPG = RPC // G                # 24 rows per group
KR = RPG + KS - 1             # 30 k/v rows per group
XR = RPC + KS - 1             # 54 x rows per core
WE = W + KS - 1               # 102 extended cols
NB = W // 16                  # 6 w-blocks
NS = RPG * NB                 # 144 sites per partition
NT = KS * KS                  # 49 taps

# feature flags
QK_PE = False     # qk d-reduce on PE (measured: strided moving too slow)
AV_PE = True      # AV tap accumulation on PE (else DVE tree)
QSUM_PE = True    # qsum on PE
DEN_PE = False    # softmax denominator on PE (f32 moving)
TAP_D = 8         # weight expansion width (16 = full, 8/4 = split mults)

_cache = {}


def _build():
    import concourse.bacc as bacc
    import concourse.bass as bass
    import concourse.tile as tile
    from concourse import mybir

    f32 = mybir.dt.float32
    f16 = mybir.dt.float16
    i32 = mybir.dt.int32
    Act = mybir.ActivationFunctionType

    nc = bacc.Bacc(
        "TRN2",
        target_bir_lowering=False,
        debug=False,
        enable_asserts=False,
        num_devices=NCORES,
    )

    xc_d = nc.dram_tensor("xc", [C + 1, XR, WE], f16, kind="ExternalInput").ap()
    wq_d = nc.dram_tensor("wq", [C + 1, C], f16, kind="ExternalInput").ap()
    wk_d = nc.dram_tensor("wk", [C + 1, C], f16, kind="ExternalInput").ap()
    wv_d = nc.dram_tensor("wv", [C + 1, C], f16, kind="ExternalInput").ap()
    rel_d = nc.dram_tensor("relv", [NT], f32, kind="ExternalInput").ap()
    eye_d = nc.dram_tensor("eyem", [128, 128], f16, kind="ExternalInput").ap()
    out_d = nc.dram_tensor("outp", [2 * C, RPG, W], f16, kind="ExternalOutput").ap()

    from contextlib import ExitStack

    with tile.TileContext(nc) as tc:
        with ExitStack() as stk:
            wp = stk.enter_context(tc.tile_pool(name="wpool", bufs=1))
            mp = stk.enter_context(tc.tile_pool(name="main", bufs=1))
            tp = stk.enter_context(tc.tile_pool(name="tmp", bufs=2))
            pp_cm = tc.tile_pool(name="psum", bufs=2, space=bass.MemorySpace.PSUM)
            pp = pp_cm.__enter__()
            xp_cm = tc.tile_pool(name="xpool", bufs=1)
            xp = xp_cm.__enter__()

            # ---- loads ----
            wq = wp.tile([C + 1, C], f16)
            wk = wp.tile([C + 1, C], f16)
            wv = wp.tile([C + 1, C], f16)
            eye = wp.tile([128, 128], f16)
            nc.sync.dma_start(out=wk, in_=wk_d)
            nc.gpsimd.dma_start(out=wq, in_=wq_d)
            xc = xp.tile([C + 1, XR, WE], f16)
            for r0, r1 in ((0, 15), (15, 30), (30, 42), (42, XR)):
                nc.sync.dma_start(out=xc[:, r0:r1, :], in_=xc_d[:, r0:r1, :])
            relsb = wp.tile([128, NT], f32)
            nc.gpsimd.dma_start(
                out=relsb, in_=rel_d.unsqueeze(0).broadcast_to([128, NT])
            )
            nc.gpsimd.dma_start(out=wv, in_=wv_d)
            nc.gpsimd.dma_start(out=eye, in_=eye_d)

            # tap permutation: even-kw first (k2o/v2o off the critical
            # path). Slot i of E/F/wn holds tap PERM[i]; relsb and k/v
            # selection use the original tap id.
            PERM = [t for t in range(NT) if (t % KS) % 2 == 0] + [
                t for t in range(NT) if (t % KS) % 2 == 1
            ]

            # ---- persistent tensors ----
            VE = WE + 10  # v tiles padded so d8 block-views stay in bounds
            k2 = mp.tile([128, KR, WE], f16)    # partition = c + 64g
            v2 = mp.tile([128, KR, VE], f16)
            k2o = mp.tile([128, KR, WE], f16)   # shifted 1 col (fp16 alignment)
            v2o = mp.tile([128, KR, VE], f16)
            q2 = mp.tile([128, RPG, W], f16)
            qs = mp.tile([128, NS], f32)
            E = mp.tile([128, NS, NT], f32, tag="bigE")   # exp(logits), tap-minor
            den = mp.tile([128, NS], f32)
            rden = mp.tile([128, NS], f32)
            wn16 = mp.tile([128, NS, NT], f16)  # normalized weights, tap-minor

            # ---- projections (as baseline) ----
            KVCH = 6
            kv_n = KR * WE // KVCH  # 510
            QCH = 6
            qrows = RPG // QCH  # 4
            qn = qrows * W  # 384

            def kv_proj(dst, wgt, evac_eng):
                for ci in range(KVCH):
                    ps = pp.tile([128, 512], f32, tag="ps_kv", name="ps")
                    for g in range(G):
                        rhs = (
                            xc[:, RPG * g : RPG * g + KR, :]
                            .rearrange("p a b -> p (a b)")[:, ci * kv_n : (ci + 1) * kv_n]
                        )
                        nc.tensor.matmul(
                            ps[64 * g : 64 * g + 64, :kv_n],
                            wgt,
                            rhs,
                            start=True,
                            stop=True,
                        )
                    rows = KR // KVCH  # 5
                    dst_sl = dst[:, ci * rows : (ci + 1) * rows, :WE]
                    ps_sl = ps[:, :kv_n].rearrange("p (a b) -> p a b", b=WE)
                    if evac_eng == "v":
                        nc.vector.tensor_copy(dst_sl, ps_sl)
                    else:
                        nc.scalar.copy(dst_sl, ps_sl)

            def shift_copy(dsto, src):
                nc.scalar.copy(dsto[:, :, : WE - 1], src[:, :, 1:WE])

            for ci in range(QCH):
                ps = pp.tile([128, 512], f32, tag="ps_q")
                for g in range(G):
                    r0 = HALO + RPG * g + ci * qrows
                    rhs = xc[:, r0 : r0 + qrows, HALO : HALO + W]
                    nc.tensor.matmul(
                        ps[64 * g : 64 * g + 64, :qn], wq, rhs, start=True, stop=True
                    )
                q2_sl = q2[:, ci * qrows : (ci + 1) * qrows, :]
                ps_sl = ps[:, :qn].rearrange("p (a b) -> p a b", b=W)
                nc.scalar.copy(q2_sl, ps_sl)
            kv_proj(k2, wk, "v")
            shift_copy(k2o, k2)
            kv_proj(v2, wv, "s")
            shift_copy(v2o, v2)

            # projections emitted; release x pool address space
            xp_cm.__exit__(None, None, None)
            abp = stk.enter_context(tc.tile_pool(name="abpool", bufs=1))

            # ---- qsum ----
            q2v = q2.rearrange("p h (a b) -> p (h a) b", b=16)  # [128, 144, 16]
            if QSUM_PE:
                ps_qs = pp.tile([128, NS], f32, tag="ps_qs", bufs=1)  # 1 bank
                for d in range(16):
                    nc.tensor.matmul(
                        ps_qs, eye, q2v[:, :, d], start=(d == 0), stop=(d == 15),
                        skip_group_check=True,
                    )
                nc.scalar.copy(qs, ps_qs)
            else:
                nc.vector.reduce_sum(out=qs, in_=q2v, axis=mybir.AxisListType.X)

            pp_cm.__exit__(None, None, None)
            app = stk.enter_context(
                tc.tile_pool(name="avpsum", bufs=1, space=bass.MemorySpace.PSUM)
            )

            # ---- F_t = exp(qs*rel_t), one op per tap (ACT) ----
            F = mp.tile([128, NS, NT], f32, tag="bigF")  # tap-minor
            for i in range(NT):
                t = PERM[i]
                nc.scalar.activation(
                    F[:, :, i], qs, Act.Exp, scale=relsb[:, t : t + 1]
                )

            # ---- qk taps ----
            def ksl(kh, kw):
                s, o = (k2, kw) if kw % 2 == 0 else (k2o, kw - 1)
                return s[:, kh : kh + RPG, o : o + W]

            if QK_PE:
                # pairs of taps; products -> 16 shifted identity-MMs -> psum
                # logits; exp(psum) -> E (fp32 in SBUF)
                for t0 in range(0, NT, 2):
                    nb = min(2, NT - t0)
                    pr = tp.tile([128, 2, RPG, W], f16, tag="pr", bufs=3)
                    for i in range(nb):
                        t = t0 + i
                        nc.vector.tensor_mul(pr[:, i], q2, ksl(t // KS, t % KS))
                    prv = pr.rearrange("p t h (a b) -> p t (h a) b", b=16)
                    psl = pp.tile([128, 2, NS], f32, tag="ps_l", bufs=4)
                    for d in range(16):
                        nc.tensor.matmul(
                            psl[:, :nb, :], eye, prv[:, :nb, :, d],
                            start=(d == 0), stop=(d == 15),
                            skip_group_check=True,
                        )
                    nc.scalar.activation(E[:, t0 : t0 + nb, :], psl[:, :nb, :], Act.Exp)
            else:
                QB = 4
                for t0 in range(0, NT, QB):
                    nb = min(QB, NT - t0)
                    pr = tp.tile([128, QB, RPG, W], f16, tag="pr4", bufs=1)
                    for i in range(nb):
                        t = PERM[t0 + i]
                        nc.vector.tensor_mul(pr[:, i], q2, ksl(t // KS, t % KS))
                    prv = pr[:, :nb].rearrange("p t h (a b) -> p t (h a) b", b=16)
                    t1 = tp.tile([128, QB, NS, 8], f16, tag="t1", bufs=1)
                    nc.vector.tensor_add(t1[:, :nb], prv[:, :, :, 0:8], prv[:, :, :, 8:16])
                    t2 = tp.tile([128, QB, NS, 4], f16, tag="t2", bufs=1)
                    nc.vector.tensor_add(t2[:, :nb], t1[:, :nb, :, 0:4], t1[:, :nb, :, 4:8])
                    t3 = tp.tile([128, QB, NS, 2], f16, tag="t3", bufs=1)
                    nc.vector.tensor_add(t3[:, :nb], t2[:, :nb, :, 0:2], t2[:, :nb, :, 2:4])
                    Aout = E[:, :, t0 : t0 + nb].rearrange("p s t -> p t s")
                    nc.vector.tensor_add(
                        Aout, t3[:, :nb, :, 0], t3[:, :nb, :, 1]
                    )
            # ---- softmax normalize (tap-minor): exp; E *= F; den; rden; wn ----
            QS4 = NS // 4
            for q0 in range(0, NS, QS4):
                Esl = E[:, q0 : q0 + QS4, :]
                nc.scalar.activation(Esl, Esl, Act.Exp)
                nc.vector.tensor_mul(Esl, Esl, F[:, q0 : q0 + QS4, :])
                nc.vector.reduce_sum(
                    out=den[:, q0 : q0 + QS4], in_=Esl, axis=mybir.AxisListType.X
                )
                nc.vector.reciprocal(rden[:, q0 : q0 + QS4], den[:, q0 : q0 + QS4])
                nc.vector.tensor_mul(
                    wn16[:, q0 : q0 + QS4, :],
                    Esl,
                    rden[:, q0 : q0 + QS4].unsqueeze(2).broadcast_to([128, QS4, NT]),
                )

            # ---- AV phase ----
            ND = 16 // TAP_D  # mults per tap
            NSTRIP = 6
            SR = RPG // NSTRIP  # rows per strip

            if AV_PE:
                avps = [
                    app.tile([128, SR, W], f32, tag=f"avps{s}", name=f"avps{s}", bufs=1)
                    for s in range(NSTRIP)
                ]

            carry = {}
            state = {"acc": None}

            def tree_push(p, level=0):
                while level in carry and level < 3:
                    prev = carry.pop(level)
                    s = abp.tile(
                        [128, RPG, W], f16,
                        tag=f"ts{level}", name=f"ts{level}",
                        bufs=3 if level == 2 else 2,
                    )
                    nc.vector.tensor_add(s, prev, p)
                    p = s
                    level += 1
                if level == 3:
                    if state["acc"] is None:
                        state["acc"] = p
                    else:
                        nc.vector.tensor_add(state["acc"], state["acc"], p)
                else:
                    carry[level] = p

            for t0 in range(0, NT, 2):
                nb = min(2, NT - t0)
                # weight expansion to TAP_D (ACT; pair 0 on DVE, its idle slot)
                wexp = abp.tile(
                    [128, 2, RPG, NB, TAP_D], f16, tag="wexp", name="wexp", bufs=3
                )
                wsl = (
                    wn16[:, :, t0 : t0 + nb]
                    .rearrange("p (h a) t -> p t h a", a=NB)
                    .unsqueeze(4)
                    .broadcast_to([128, nb, RPG, NB, TAP_D])
                )
                if t0 == 0:
                    nc.vector.tensor_copy(wexp[:, :nb], wsl)
                else:
                    nc.scalar.copy(wexp[:, :nb], wsl)
                for i in range(nb):
                    slot = t0 + i
                    t = PERM[slot]
                    kh, kw = t // KS, t % KS
                    vsrc, kwoff = (v2, kw) if kw % 2 == 0 else (v2o, kw - 1)
                    p = abp.tile([128, RPG, NB, 16], f16, tag="avp", name="avp", bufs=3)
                    for j in range(ND):
                        c0 = kwoff + j * TAP_D
                        vview = (
                            vsrc[:, kh : kh + RPG, c0 : c0 + 16 * NB]
                            .rearrange("p h (a b) -> p h a b", b=16)[:, :, :, 0:TAP_D]
                        )
                        nc.vector.tensor_mul(
                            p[:, :, :, j * TAP_D : (j + 1) * TAP_D],
                            wexp[:, i],
                            vview,
                        )
                    if AV_PE:
                        pv = p.rearrange("p h a b -> p h (a b)")
                        for s in range(NSTRIP):
                            nc.tensor.matmul(
                                avps[s], eye,
                                pv[:, s * SR : (s + 1) * SR, :],
                                start=(slot == 0), stop=(slot == NT - 1),
                                skip_group_check=True,
                            )
                    else:
                        tree_push(p.rearrange("p h a b -> p h (a b)"))

            if AV_PE:
                for s in range(NSTRIP):
                    oute = abp.tile([128, SR, W], f16, tag="oute", name="oute", bufs=3)
                    if s % 2 == 1:
                        nc.vector.tensor_scalar_max(oute, avps[s], 0.0)
                    else:
                        nc.scalar.activation(oute, avps[s], Act.Relu)
                    eng = nc.sync if s % 2 == 0 else nc.gpsimd
                    eng.dma_start(
                        out=out_d[:, s * SR : (s + 1) * SR, :], in_=oute
                    )
            else:
                acc = state["acc"]
                for lv in sorted(carry):
                    nc.vector.tensor_add(acc, acc, carry.pop(lv))
                oute = mp.tile([128, RPG, W], f16, tag="oute_f")
                qt = RPG // 4
                for r0 in range(0, RPG, qt):
                    nc.scalar.activation(
                        oute[:, r0 : r0 + qt, :], acc[:, r0 : r0 + qt, :], Act.Relu
                    )
                    nc.sync.dma_start(
                        out=out_d[:, r0 : r0 + qt, :], in_=oute[:, r0 : r0 + qt, :]
                    )

    nc.compile()
    return nc


def _get_nc():
    if "nc" not in _cache:
        _cache["nc"] = _build()
    return _cache["nc"]


def _prep_inputs(inputs):
    """Host-side shard prep. Returns list of 8 in_maps."""
    x = np.ascontiguousarray(np.asarray(inputs["input_x"], dtype=np.float32))
    qw = np.asarray(inputs["q_w"], np.float32)
    qb = np.asarray(inputs["q_b"], np.float32)
    kw_ = np.asarray(inputs["k_w"], np.float32)
    kb = np.asarray(inputs["k_b"], np.float32)
    vw = np.asarray(inputs["v_w"], np.float32)
    vb = np.asarray(inputs["v_b"], np.float32)
    rh = np.asarray(inputs["rel_h"], np.float32).sum(0)[:, 0]  # (7,)
    rw = np.asarray(inputs["rel_w"], np.float32).sum(0)[0, :]  # (7,)

    wq = np.concatenate([qw.T, qb[None, :]], axis=0).astype(np.float16)  # (65, 64)
    wk = np.concatenate([kw_.T, kb[None, :]], axis=0).astype(np.float16)
    wv = np.concatenate([vw.T, vb[None, :]], axis=0).astype(np.float16)
    relv = (rh[:, None] + rw[None, :]).reshape(-1).astype(np.float32)  # (49,)
    eyem = np.eye(128, dtype=np.float16)

    xpad = np.zeros((B, C + 1, H + 2 * HALO, W + 2 * HALO), np.float16)
    xpad[:, :C, HALO : HALO + H, HALO : HALO + W] = x
    xpad[:, C, :, :] = 1.0

    in_maps = []
    for j in range(NCORES):
        b = j // 2
        r0 = RPC * (j % 2)
        xc = np.ascontiguousarray(xpad[b, :, r0 : r0 + XR, :])  # (65, 54, 102)
        in_maps.append(
            {"xc": xc, "wq": wq, "wk": wk, "wv": wv, "relv": relv, "eyem": eyem}
        )
    return in_maps


def _assemble(results):
    y = np.empty((B, C, H, W), np.float32)
    for j in range(NCORES):
        o = results[j]["outp"]
        b = j // 2
        r0 = RPC * (j % 2)
        for g in range(G):
            y[b, :, r0 + RPG * g : r0 + RPG * (g + 1), :] = o[64 * g : 64 * g + 64]
    return y


def _install_ntff_hook():
    import types
    import antenv

    if "antenv.axon_hooks" in sys.modules:
        return
    mod = types.ModuleType("antenv.axon_hooks")
    _state = {"hook": None}
    mod.set_axon_ntff_profile_hook = lambda h: _state.__setitem__("hook", h)
    mod.get_axon_ntff_profile_hook = lambda: _state["hook"]
    sys.modules["antenv.axon_hooks"] = mod
    antenv.axon_hooks = mod
    from trn_agent_boot.trn_boot import _ntff_profile_via_ctypes

    mod.set_axon_ntff_profile_hook(_ntff_profile_via_ctypes("/opt/axon/libaxon_pjrt.so"))
    from concourse import bass_utils

    bass_utils.upload_artifacts = lambda tmpdir: tmpdir


def kernel(**inputs) -> np.ndarray:
    from concourse import bass_utils

    nc = _get_nc()
    in_maps = _prep_inputs(inputs)
    trace = bool(int(os.environ.get("KERNEL_TRACE", "0")))
    kw = {}
    if trace:
        _install_ntff_hook()
        kw["tmpdir"] = os.environ.get("KERNEL_TRACE_DIR") or None
    res = bass_utils.run_bass_kernel_spmd(
        nc, in_maps, core_ids=list(range(NCORES)), trace=trace, **kw
    )
    _cache["last_result"] = res
    return _assemble(res.results)


def kernel_sim(inputs, cores=(0,)):
    from concourse.bass_interp import CoreSim

    nc = _get_nc()
    in_maps = _prep_inputs(inputs)
    outs = {}
    for j in cores:
        sim = CoreSim(nc, trace=False, require_finite=True, require_nnan=True)
        for name, arr in in_maps[j].items():
            sim.tensor(name)[:] = arr
        sim.simulate(check_with_hw=False)
        outs[j] = np.array(sim.tensor("outp"))
    return outs


# revision 5
# speedup vs baseline: 1.2173x; 1.0185x over previous
"""Trainium2 Bass kernel for nn_AttentionBlock (sparse 7x7 windowed per-channel
attention), v2: PE-offloaded reductions.

Semantics (validated vs reference): the torch-faithful scrambled reshape makes
this, in original coordinates, a per-(b, c, h, w-block-of-16) attention:
  logits[kh,kw] = sum_{d<16} q[c,h,16w0+d] * kpad[c,h+kh,16w0+d+kw]
                  + qsum*(rh[kh]+rw[kw])
  out[c,h,16w0+d] = relu( sum_{kh,kw} softmax(logits)[kh,kw]
                          * vpad[c,h+kh,16w0+d+kw] )
where kpad/vpad = conv1x1(x)+bias inside the image and exactly bias in the pad
border (conv of zero-padded x reproduces this).

Sharding: pure data parallel over 8 cores: core j -> batch j//2, image rows
[48*(j%2), 48*(j%2)+48). Each core packs its 48 rows as 2 partition-groups of
24 rows (partition = c + 64*g) so elementwise work uses all 128 partitions.

Engine split (the kernel is elementwise-bound; binary tensor ops only run on
the DVE, at 2 elem/cycle fp16 max, so the wins come from moving every
reduction it can spare onto other engines):
  DVE:  49 qk tap mults (fp16 2x) + 4-level pairwise tree -> fp32 logits
        (tap-minor layout so the den reduce is contiguous); softmax muls
        E*=F, wn=E*rden; AV mults in d=8 halves (fp16 2x).
  PE:   q/k/v 1x1-conv matmuls; qsum via 16 shifted identity-matmuls with
        PSUM accumulation; AV tap-sum via 6 row-strip identity-matmuls per
        tap accumulating all 49 weighted products in PSUM (measured
        ~0.5ns/col for contiguous moving; LDWEIGHTS pipelines away).
        Strided-moving matmuls measured 4.3x slower, so the qk d-reduce
        stays on the DVE tree; fp16 tensor_reduce measured 1x - no good.
  ACT:  weight-broadcast expansion to d=8 (stride-0 input forces 1x, so
        half-width halves the cost; the paired DVE mults stay 2x via
        4B-aligned block views into 112-col-padded v tiles); exp; F_t =
        exp(qs*rel_t) (per-partition scale carries rel_t); psum evacs;
        relu on even strips (odd strips relu on DVE to parallelize the
        drain).
Taps are processed even-kw-first (PERM) so the odd-shifted k2o/v2o copies
(fp16 2x alignment for odd kw) are off the critical path; E/F/wn slots are
permutation-indexed. Input DMA is split across the Sync and GpSimd (SWDGE)
queues, x ahead of the later-needed weights. fp16 quantization validated
end-to-end: rel err ~2.6e-3 vs 2e-2 tolerance (HW); ~322-385us baseline ->
~262us measured (run-to-run device clock varies ~20%).
"""

import sys
import os

sys.path.insert(0, "/opt/trn_rl_repo")

import numpy as np

B, C, H, W = 4, 64, 96, 96
KS, NH = 7, 4
HALO = (KS - 1) // 2          # 3
NCORES = 8
RPC = H // 2                  # 48 rows per core
G = 2                         # partition groups per core
RPG = RPC // G                # 24 rows per group
KR = RPG + KS - 1             # 30 k/v rows per group
XR = RPC + KS - 1             # 54 x rows per core
WE = W + KS - 1               # 102 extended cols
NB = W // 16                  # 6 w-blocks
NS = RPG * NB                 # 144 sites per partition
NT = KS * KS                  # 49 taps

# feature flags
QK_PE = False     # qk d-reduce on PE (measured: strided moving too slow)
AV_PE = True      # AV tap accumulation on PE (else DVE tree)
QSUM_PE = True    # qsum on PE
DEN_PE = False    # softmax denominator on PE (f32 moving)
TAP_D = 8         # weight expansion width (16 = full, 8/4 = split mults)

_cache = {}


def _build():
    import concourse.bacc as bacc
    import concourse.bass as bass
    import concourse.tile as tile
    from concourse import mybir

    f32 = mybir.dt.float32
    f16 = mybir.dt.float16
    i32 = mybir.dt.int32
    Act = mybir.ActivationFunctionType

    nc = bacc.Bacc(
        "TRN2",
        target_bir_lowering=False,
        debug=False,
        enable_asserts=False,
        num_devices=NCORES,
    )

    xc_d = nc.dram_tensor("xc", [C + 1, XR, WE], f16, kind="ExternalInput").ap()
    wq_d = nc.dram_tensor("wq", [C + 1, C], f16, kind="ExternalInput").ap()
    wk_d = nc.dram_tensor("wk", [C + 1, C], f16, kind="ExternalInput").ap()
    wv_d = nc.dram_tensor("wv", [C + 1, C], f16, kind="ExternalInput").ap()
    rel_d = nc.dram_tensor("relv", [NT], f32, kind="ExternalInput").ap()
    eye_d = nc.dram_tensor("eyem", [128, 128], f16, kind="ExternalInput").ap()
    out_d = nc.dram_tensor("outp", [2 * C, RPG, W], f16, kind="ExternalOutput").ap()

    from contextlib import ExitStack

    with tile.TileContext(nc) as tc:
        with ExitStack() as stk:
            wp = stk.enter_context(tc.tile_pool(name="wpool", bufs=1))
            mp = stk.enter_context(tc.tile_pool(name="main", bufs=1))
            tp = stk.enter_context(tc.tile_pool(name="tmp", bufs=2))
            pp_cm = tc.tile_pool(name="psum", bufs=2, space=bass.MemorySpace.PSUM)
            pp = pp_cm.__enter__()
            xp_cm = tc.tile_pool(name="xpool", bufs=1)
            xp = xp_cm.__enter__()

            # ---- loads ----
            wq = wp.tile([C + 1, C], f16)
            wk = wp.tile([C + 1, C], f16)
            wv = wp.tile([C + 1, C], f16)
            eye = wp.tile([128, 128], f16)
            nc.sync.dma_start(out=wk, in_=wk_d)
            nc.gpsimd.dma_start(out=wq, in_=wq_d)
            xc = xp.tile([C + 1, XR, WE], f16)
            for r0, r1 in ((0, 15), (15, 30), (30, 42), (42, XR)):
                nc.sync.dma_start(out=xc[:, r0:r1, :], in_=xc_d[:, r0:r1, :])
            relsb = wp.tile([128, NT], f32)
            nc.gpsimd.dma_start(
                out=relsb, in_=rel_d.unsqueeze(0).broadcast_to([128, NT])
            )
            nc.gpsimd.dma_start(out=wv, in_=wv_d)
            nc.gpsimd.dma_start(out=eye, in_=eye_d)

            # tap permutation: even-kw first (k2o/v2o off the critical
            # path). Slot i of E/F/wn holds tap PERM[i]; relsb and k/v
            # selection use the original tap id.
            PERM = [t for t in range(NT) if (t % KS) % 2 == 0] + [
                t for t in range(NT) if (t % KS) % 2 == 1
            ]

            # ---- persistent tensors ----
            VE = WE + 10  # v tiles padded so d8 block-views stay in bounds
            k2 = mp.tile([128, KR, WE], f16)    # partition = c + 64g
            v2 = mp.tile([128, KR, VE], f16)
            k2o = mp.tile([128, KR, WE], f16)   # shifted 1 col (fp16 alignment)
            v2o = mp.tile([128, KR, VE], f16)
            q2 = mp.tile([128, RPG, W], f16)
            qs = mp.tile([128, NS], f32)
            E = mp.tile([128, NS, NT], f32, tag="bigE")   # exp(logits), tap-minor
            den = mp.tile([128, NS], f32)
            rden = mp.tile([128, NS], f32)
            wn16 = mp.tile([128, NS, NT], f16)  # normalized weights, tap-minor

            # ---- projections (as baseline) ----
            KVCH = 6
            kv_n = KR * WE // KVCH  # 510
            QCH = 6
            qrows = RPG // QCH  # 4
            qn = qrows * W  # 384

            def kv_proj(dst, wgt, evac_eng):
                for ci in range(KVCH):
                    ps = pp.tile([128, 512], f32, tag="ps_kv", name="ps")
                    for g in range(G):
                        rhs = (
                            xc[:, RPG * g : RPG * g + KR, :]
                            .rearrange("p a b -> p (a b)")[:, ci * kv_n : (ci + 1) * kv_n]
                        )
                        nc.tensor.matmul(
                            ps[64 * g : 64 * g + 64, :kv_n],
                            wgt,
                            rhs,
                            start=True,
                            stop=True,
                        )
                    rows = KR // KVCH  # 5
                    dst_sl = dst[:, ci * rows : (ci + 1) * rows, :WE]
                    ps_sl = ps[:, :kv_n].rearrange("p (a b) -> p a b", b=WE)
                    if evac_eng == "v":
                        nc.vector.tensor_copy(dst_sl, ps_sl)
                    else:
                        nc.scalar.copy(dst_sl, ps_sl)

            def shift_copy(dsto, src):
                nc.scalar.copy(dsto[:, :, : WE - 1], src[:, :, 1:WE])

            for ci in range(QCH):
                ps = pp.tile([128, 512], f32, tag="ps_q")
                for g in range(G):
                    r0 = HALO + RPG * g + ci * qrows
                    rhs = xc[:, r0 : r0 + qrows, HALO : HALO + W]
                    nc.tensor.matmul(
                        ps[64 * g : 64 * g + 64, :qn], wq, rhs, start=True, stop=True
                    )
                q2_sl = q2[:, ci * qrows : (ci + 1) * qrows, :]
                ps_sl = ps[:, :qn].rearrange("p (a b) -> p a b", b=W)
                nc.scalar.copy(q2_sl, ps_sl)
            kv_proj(k2, wk, "v")
            shift_copy(k2o, k2)
            kv_proj(v2, wv, "s")
            shift_copy(v2o, v2)

            # projections emitted; release x pool address space
            xp_cm.__exit__(None, None, None)
            abp = stk.enter_context(tc.tile_pool(name="abpool", bufs=1))

            # ---- qsum ----
            q2v = q2.rearrange("p h (a b) -> p (h a) b", b=16)  # [128, 144, 16]
            if QSUM_PE:
                ps_qs = pp.tile([128, NS], f32, tag="ps_qs", bufs=1)  # 1 bank
                for d in range(16):
                    nc.tensor.matmul(
                        ps_qs, eye, q2v[:, :, d], start=(d == 0), stop=(d == 15),
                        skip_group_check=True,
                    )
                nc.scalar.copy(qs, ps_qs)
            else:
                nc.vector.reduce_sum(out=qs, in_=q2v, axis=mybir.AxisListType.X)

            pp_cm.__exit__(None, None, None)
            app = stk.enter_context(
                tc.tile_pool(name="avpsum", bufs=1, space=bass.MemorySpace.PSUM)
            )

            # ---- F_t = exp(qs*rel_t), one op per tap (ACT) ----
            F = mp.tile([128, NS, NT], f32, tag="bigF")  # tap-minor
            for i in range(NT):
                t = PERM[i]
                nc.scalar.activation(
                    F[:, :, i], qs, Act.Exp, scale=relsb[:, t : t + 1]
                )

            # ---- qk taps ----
            def ksl(kh, kw):
                s, o = (k2, kw) if kw % 2 == 0 else (k2o, kw - 1)
                return s[:, kh : kh + RPG, o : o + W]

            if QK_PE:
                # pairs of taps; products -> 16 shifted identity-MMs -> psum
                # logits; exp(psum) -> E (fp32 in SBUF)
                for t0 in range(0, NT, 2):
                    nb = min(2, NT - t0)
                    pr = tp.tile([128, 2, RPG, W], f16, tag="pr", bufs=3)
                    for i in range(nb):
                        t = t0 + i
                        nc.vector.tensor_mul(pr[:, i], q2, ksl(t // KS, t % KS))
                    prv = pr.rearrange("p t h (a b) -> p t (h a) b", b=16)
                    psl = pp.tile([128, 2, NS], f32, tag="ps_l", bufs=4)
                    for d in range(16):
                        nc.tensor.matmul(
                            psl[:, :nb, :], eye, prv[:, :nb, :, d],
                            start=(d == 0), stop=(d == 15),
                            skip_group_check=True,
                        )
                    nc.scalar.activation(E[:, t0 : t0 + nb, :], psl[:, :nb, :], Act.Exp)
            else:
                QB = 4
                for t0 in range(0, NT, QB):
                    nb = min(QB, NT - t0)
                    pr = tp.tile([128, QB, RPG, W], f16, tag="pr4", bufs=1)
                    for i in range(nb):
                        t = PERM[t0 + i]
                        nc.vector.tensor_mul(pr[:, i], q2, ksl(t // KS, t % KS))
                    prv = pr[:, :nb].rearrange("p t h (a b) -> p t (h a) b", b=16)
                    t1 = tp.tile([128, QB, NS, 8], f16, tag="t1", bufs=1)
                    nc.vector.tensor_add(t1[:, :nb], prv[:, :, :, 0:8], prv[:, :, :, 8:16])
                    t2 = tp.tile([128, QB, NS, 4], f16, tag="t2", bufs=1)
                    nc.vector.tensor_add(t2[:, :nb], t1[:, :nb, :, 0:4], t1[:, :nb, :, 4:8])
                    t3 = tp.tile([128, QB, NS, 2], f16, tag="t3", bufs=1)
                    nc.vector.tensor_add(t3[:, :nb], t2[:, :nb, :, 0:2], t2[:, :nb, :, 2:4])
                    Aout = E[:, :, t0 : t0 + nb].rearrange("p s t -> p t s")
                    nc.vector.tensor_add(
                        Aout, t3[:, :nb, :, 0], t3[:, :nb, :, 1]
                    )
            # ---- softmax normalize (tap-minor): exp; E *= F; den; rden; wn ----
            QS4 = NS // 4
            for q0 in range(0, NS, QS4):
                Esl = E[:, q0 : q0 + QS4, :]
                nc.scalar.activation(Esl, Esl, Act.Exp)
                nc.vector.tensor_mul(Esl, Esl, F[:, q0 : q0 + QS4, :])
                nc.vector.reduce_sum(
                    out=den[:, q0 : q0 + QS4], in_=Esl, axis=mybir.AxisListType.X
                )
                nc.vector.reciprocal(rden[:, q0 : q0 + QS4], den[:, q0 : q0 + QS4])
                nc.vector.tensor_mul(
                    wn16[:, q0 : q0 + QS4, :],
                    Esl,
                    rden[:, q0 : q0 + QS4].unsqueeze(2).broadcast_to([128, QS4, NT]),
                )

            # ---- AV phase ----
            ND = 16 // TAP_D  # mults per tap
            NSTRIP = 6
            SR = RPG // NSTRIP  # rows per strip

            if AV_PE:
                avps = [
                    app.tile([128, SR, W], f32, tag=f"avps{s}", name=f"avps{s}", bufs=1)
                    for s in range(NSTRIP)
                ]

            carry = {}
            state = {"acc": None}

            def tree_push(p, level=0):
                while level in carry and level < 3:
                    prev = carry.pop(level)
                    s = abp.tile(
                        [128, RPG, W], f16,
                        tag=f"ts{level}", name=f"ts{level}",
                        bufs=3 if level == 2 else 2,
                    )
                    nc.vector.tensor_add(s, prev, p)
                    p = s
                    level += 1
                if level == 3:
                    if state["acc"] is None:
                        state["acc"] = p
                    else:
                        nc.vector.tensor_add(state["acc"], state["acc"], p)
                else:
                    carry[level] = p

            for t0 in range(0, NT, 2):
                nb = min(2, NT - t0)
                # weight expansion to TAP_D (ACT; pair 0 on DVE, its idle slot)
                wexp = abp.tile(
                    [128, 2, RPG, NB, TAP_D], f16, tag="wexp", name="wexp", bufs=3
                )
                wsl = (
                    wn16[:, :, t0 : t0 + nb]
                    .rearrange("p (h a) t -> p t h a", a=NB)
                    .unsqueeze(4)
                    .broadcast_to([128, nb, RPG, NB, TAP_D])
                )
                if t0 == 0:
                    nc.vector.tensor_copy(wexp[:, :nb], wsl)
                else:
                    nc.scalar.copy(wexp[:, :nb], wsl)
                for i in range(nb):
                    slot = t0 + i
                    t = PERM[slot]
                    kh, kw = t // KS, t % KS
                    vsrc, kwoff = (v2, kw) if kw % 2 == 0 else (v2o, kw - 1)
                    p = abp.tile([128, RPG, NB, 16], f16, tag="avp", name="avp", bufs=3)
                    for j in range(ND):
                        c0 = kwoff + j * TAP_D
                        vview = (
                            vsrc[:, kh : kh + RPG, c0 : c0 + 16 * NB]
                            .rearrange("p h (a b) -> p h a b", b=16)[:, :, :, 0:TAP_D]
                        )
                        nc.vector.tensor_mul(
                            p[:, :, :, j * TAP_D : (j + 1) * TAP_D],
                            wexp[:, i],
                            vview,
                        )
                    if AV_PE:
                        pv = p.rearrange("p h a b -> p h (a b)")
                        for s in range(NSTRIP):
                            nc.tensor.matmul(
                                avps[s], eye,
                                pv[:, s * SR : (s + 1) * SR, :],
                                start=(slot == 0), stop=(slot == NT - 1),
                                skip_group_check=True,
                            )
                    else:
                        tree_push(p.rearrange("p h a b -> p h (a b)"))

            if AV_PE:
                for s in range(NSTRIP):
                    oute = abp.tile([128, SR, W], f16, tag="oute", name="oute", bufs=3)
                    if s % 2 == 1:
                        nc.vector.tensor_scalar_max(oute, avps[s], 0.0)
                    else:
                        nc.scalar.activation(oute, avps[s], Act.Relu)
                    nc.sync.dma_start(
                        out=out_d[:, s * SR : (s + 1) * SR, :], in_=oute
                    )
            else:
                acc = state["acc"]
                for lv in sorted(carry):
                    nc.vector.tensor_add(acc, acc, carry.pop(lv))
                oute = mp.tile([128, RPG, W], f16, tag="oute_f")
                qt = RPG // 4
                for r0 in range(0, RPG, qt):
                    nc.scalar.activation(
                        oute[:, r0 : r0 + qt, :], acc[:, r0 : r0 + qt, :], Act.Relu
                    )
                    nc.sync.dma_start(
                        out=out_d[:, r0 : r0 + qt, :], in_=oute[:, r0 : r0 + qt, :]
                    )

    nc.compile()
    return nc


def _get_nc():
    if "nc" not in _cache:
        _cache["nc"] = _build()
    return _cache["nc"]


def _prep_inputs(inputs):
    """Host-side shard prep. Returns list of 8 in_maps."""
    x = np.ascontiguousarray(np.asarray(inputs["input_x"], dtype=np.float32))
    qw = np.asarray(inputs["q_w"], np.float32)
    qb = np.asarray(inputs["q_b"], np.float32)
    kw_ = np.asarray(inputs["k_w"], np.float32)
    kb = np.asarray(inputs["k_b"], np.float32)
    vw = np.asarray(inputs["v_w"], np.float32)
    vb = np.asarray(inputs["v_b"], np.float32)
    rh = np.asarray(inputs["rel_h"], np.float32).sum(0)[:, 0]  # (7,)
    rw = np.asarray(inputs["rel_w"], np.float32).sum(0)[0, :]  # (7,)

    wq = np.concatenate([qw.T, qb[None, :]], axis=0).astype(np.float16)  # (65, 64)
    wk = np.concatenate([kw_.T, kb[None, :]], axis=0).astype(np.float16)
    wv = np.concatenate([vw.T, vb[None, :]], axis=0).astype(np.float16)
    relv = (rh[:, None] + rw[None, :]).reshape(-1).astype(np.float32)  # (49,)
    eyem = np.eye(128, dtype=np.float16)

    xpad = np.zeros((B, C + 1, H + 2 * HALO, W + 2 * HALO), np.float16)
    xpad[:, :C, HALO : HALO + H, HALO : HALO + W] = x
    xpad[:, C, :, :] = 1.0

    in_maps = []
    for j in range(NCORES):
        b = j // 2
        r0 = RPC * (j % 2)
        xc = np.ascontiguousarray(xpad[b, :, r0 : r0 + XR, :])  # (65, 54, 102)
        in_maps.append(
            {"xc": xc, "wq": wq, "wk": wk, "wv": wv, "relv": relv, "eyem": eyem}
        )
    return in_maps


def _assemble(results):
    y = np.empty((B, C, H, W), np.float32)
    for j in range(NCORES):
        o = results[j]["outp"]
        b = j // 2
        r0 = RPC * (j % 2)
        for g in range(G):
            y[b, :, r0 + RPG * g : r0 + RPG * (g + 1), :] = o[64 * g : 64 * g + 64]
    return y


def _install_ntff_hook():
    import types
    import antenv

    if "antenv.axon_hooks" in sys.modules:
        return
    mod = types.ModuleType("antenv.axon_hooks")
    _state = {"hook": None}
    mod.set_axon_ntff_profile_hook = lambda h: _state.__setitem__("hook", h)
    mod.get_axon_ntff_profile_hook = lambda: _state["hook"]
    sys.modules["antenv.axon_hooks"] = mod
    antenv.axon_hooks = mod
    from trn_agent_boot.trn_boot import _ntff_profile_via_ctypes

    mod.set_axon_ntff_profile_hook(_ntff_profile_via_ctypes("/opt/axon/libaxon_pjrt.so"))
    from concourse import bass_utils

    bass_utils.upload_artifacts = lambda tmpdir: tmpdir


def kernel(**inputs) -> np.ndarray:
    from concourse import bass_utils

    nc = _get_nc()
    in_maps = _prep_inputs(inputs)
    trace = bool(int(os.environ.get("KERNEL_TRACE", "0")))
    kw = {}
    if trace:
        _install_ntff_hook()
        kw["tmpdir"] = os.environ.get("KERNEL_TRACE_DIR") or None
    res = bass_utils.run_bass_kernel_spmd(
        nc, in_maps, core_ids=list(range(NCORES)), trace=trace, **kw
    )
    _cache["last_result"] = res
    return _assemble(res.results)


def kernel_sim(inputs, cores=(0,)):
    from concourse.bass_interp import CoreSim

    nc = _get_nc()
    in_maps = _prep_inputs(inputs)
    outs = {}
    for j in cores:
        sim = CoreSim(nc, trace=False, require_finite=True, require_nnan=True)
        for name, arr in in_maps[j].items():
            sim.tensor(name)[:] = arr
        sim.simulate(check_with_hw=False)
        outs[j] = np.array(sim.tensor("outp"))
    return outs
